# revision 1
# baseline (speedup 1.0000x reference)
"""Distributed multi-head attention kernel for 8 TRN2 NeuronCores.

Problem: B=2, N=2048, C=1024, H=16 heads, D=64.
  out = softmax((q@Wq)(k@Wk)^T / sqrt(D)) @ (v@Wv) @ Wo   (per head, biases zero)

Sharding: batch x head-group.  Core c owns batch b=c//4 and head group
g=c%4 -> heads [4g, 4g+4) = channel block [256g, 256g+256).
Zero-redundancy: each core projects only its own 256 Q/K/V channels for
its batch, runs attention for its 4 heads over all 2048 queries/keys,
and computes the row-sharded out-proj partial out^T = Wo_s^T @ A^T
(bf16).  The host sums the 4 partials per batch (the "all-reduce" of
the sharding hint, done at gather time) -- no device collectives.

Per-core engine budget (measured): PE ~210us active (matmul columns
164us @2.4GHz -- scores and PV are stream-bound at D=64 -- plus
ldweights/p-state tax), ScalarE 142.6us of exp (128 x [128,1024]
ACTIVATE @ ~1114ns), ~14MB input DMA.  The kernel is PE-bound, so the
schedule keeps the PE streaming from ~12us on and ScalarE as close
behind as the data deadlines allow:

  - inputs arrive as batched 3D-AP DMAs (512-column quarters for
    xq/xk) split across both hardware DGE queues (SP: q/v-side + wo;
    Activation: k-side) so the ~3MB the first score pairs need lands
    first; one long junk-matmul ACCUMULATION group (no per-matmul
    semaphores) warms the PE clock during the wait.
  - query-block-major pipeline with one "slot" per score group (one
    exp, ~1.1us).  Per 512-query block qb, 4 heads x 8 score pairs
    stream into ScalarE; PV of lagging heads, out-proj of qb-1, and
    the Q projection of qb+1 ride pair-by-pair inside the slots.
    The last head's PV is slot-lagged by one pair so only one pair +
    normalize + out-proj remain after the final exp.
  - PV is V'-stationary (65 weight cols; col 64 = ones accumulates
    the softmax denominator in psum row 64).  A P-stationary PV costs
    +214us in serial 128-col weight reloads -- measured.
  - normalize: copy denom row -> reciprocal -> gpsimd partition
    broadcast -> multiply, writing A^T planes directly (no transposes).
  - SBUF: x tensors live on the right-side allocator stack, released
    LIFO (xk -> xv -> xq) so the 50KB P pool fits.
  - PSUM: 4 banks of score groups (2x[128,1024]) + 2 PV banks +
    2 shared banks for qk/v/out-proj groups = exactly 8.

Measured 244-246us (vs 296.9us baseline) on a cold device; the
device clocks throttle ~1.2x under repeated back-to-back runs.
"""

import sys

sys.path.insert(0, "/opt/trn_rl_repo")

from contextlib import ExitStack

import numpy as np
import ml_dtypes

import concourse.bass as bass
import concourse.bacc as bacc
import concourse.mybir as mybir
import concourse.tile as tile
from concourse.bass_utils import run_bass_kernel_spmd

BF16 = mybir.dt.bfloat16
F32 = mybir.dt.float32
Exp = mybir.ActivationFunctionType.Exp

B, N, C = 2, 2048, 1024
H, D = 16, 64
HC = 4              # heads per core
CB = HC * D         # own channel block = 256
DV = D + 1          # V cols per head incl. ones column
NCHUNK = N // 128   # 16 key chunks
NQB = N // 512      # 4 query blocks
SCALE = 1.0 / np.sqrt(D)

_CACHE = {}


def build_nc():
    nc = bacc.Bacc("TRN2", target_bir_lowering=False, debug=False, num_devices=8)

    xqT = nc.declare_dram_parameter("xqT", [C, N], BF16, isOutput=False)
    xkT = nc.declare_dram_parameter("xkT", [C, N], BF16, isOutput=False)
    xvT = nc.declare_dram_parameter("xvT", [C, N], BF16, isOutput=False)
    wq = nc.declare_dram_parameter("wq", [C, CB], BF16, isOutput=False)
    wk = nc.declare_dram_parameter("wk", [C, CB], BF16, isOutput=False)
    wv = nc.declare_dram_parameter("wv", [C, CB], BF16, isOutput=False)
    wo = nc.declare_dram_parameter("wo", [CB, C], BF16, isOutput=False)
    outT = nc.declare_dram_parameter("outT", [C, N], BF16, isOutput=True)

    with tile.TileContext(nc) as tc, ExitStack() as top:
        # ---------------- resident SBUF ----------------
        res = top.enter_context(tc.tile_pool(name="res", bufs=1))
        # Q^T / K^T: plane p holds head 2p in rows 0:64, head 2p+1 in 64:128
        qT_sb = res.tile([128, 2 * N], BF16, tag="qT")
        kT_sb = res.tile([128, 2 * N], BF16, tag="kT")
        # V' is 65 cols per (kc, h): col 64 = ones so the PV matmul
        # accumulates the softmax denominator in psum row 64.
        v1_sb = res.tile([128, NCHUNK * HC * DV], BF16, tag="v1")
        aT0_sb = res.tile([128, N], BF16, tag="aT0")   # A^T rows 0:128 (h 0,1)
        aT1_sb = res.tile([128, N], BF16, tag="aT1")   # A^T rows 128:256 (h 2,3)
        draw_sb = res.tile([1, 512], F32, tag="draw")
        drow_sb = res.tile([1, 512], F32, tag="drow")

        def q_slice(h, qb):
            base = N * (h // 2)
            return qT_sb[64 * (h % 2):64 * (h % 2) + 64,
                         base + 512 * qb:base + 512 * (qb + 1)]

        def k_slice(h, kc):
            base = N * (h // 2)
            return kT_sb[64 * (h % 2):64 * (h % 2) + 64,
                         base + 128 * kc:base + 128 * (kc + 1)]

        v3 = v1_sb[:].rearrange("p (kc h x) -> p kc h x", kc=NCHUNK, x=DV)

        # ---------------- pools ----------------
        main = ExitStack()
        wpool = main.enter_context(tc.tile_pool(name="wpool", bufs=4))
        P_pool = main.enter_context(tc.tile_pool(name="P_pool", bufs=25))
        dpool = main.enter_context(tc.tile_pool(name="dpool", bufs=2))
        ospool = main.enter_context(tc.tile_pool(name="ospool", bufs=3))
        spool = main.enter_context(
            tc.tile_pool(name="spool", bufs=2, space="PSUM"))   # 2x2 banks
        pvpool = main.enter_context(
            tc.tile_pool(name="pvpool", bufs=2, space="PSUM"))  # 2x1 banks
        gpool = main.enter_context(
            tc.tile_pool(name="gpool", bufs=2, space="PSUM"))   # 2x1 banks
        xq_stack = ExitStack()
        xqpool = xq_stack.enter_context(
            tc.tile_pool(name="xqpool", bufs=4, side="right"))
        xv_stack = ExitStack()
        xvpool = xv_stack.enter_context(
            tc.tile_pool(name="xvpool", bufs=2, side="right"))
        xk_stack = ExitStack()
        xkpool = xk_stack.enter_context(
            tc.tile_pool(name="xkpool", bufs=4, side="right"))

        # -------- input DMA: one batched transfer per half-tensor --------
        # DRAM [1024, n] viewed as [128 partitions, 8 cc-chunks, n].
        def dram3(t, lo, hi):
            return t[:].rearrange("(c p) n -> p c n", p=128)[:, :, lo:hi]

        wq_t = res.tile([128, 8 * CB], BF16, tag="wqt")
        wk_t = res.tile([128, 8 * CB], BF16, tag="wkt")
        wv_t = res.tile([128, 8 * CB], BF16, tag="wvt")
        wo_t = res.tile([128, 2 * C], BF16, tag="wot")
        wq3 = wq_t[:].rearrange("p (c n) -> p c n", c=8)
        wk3 = wk_t[:].rearrange("p (c n) -> p c n", c=8)
        wv3 = wv_t[:].rearrange("p (c n) -> p c n", c=8)
        wo3 = wo_t[:].rearrange("p (j n) -> p j n", j=2)

        xq_t = [xqpool.tile([128, 8 * 512], BF16, tag="xq", name=f"xq{i}")
                for i in range(4)]
        xk_t = [xkpool.tile([128, 8 * 512], BF16, tag="xk", name=f"xk{i}")
                for i in range(4)]
        xv_t = [xvpool.tile([128, 8 * 1024], BF16, tag="xv", name=f"xv{i}")
                for i in range(2)]
        xq3 = [t[:].rearrange("p (c n) -> p c n", c=8) for t in xq_t]
        xk3 = [t[:].rearrange("p (c n) -> p c n", c=8) for t in xk_t]
        xv3 = [t[:].rearrange("p (c n) -> p c n", c=8) for t in xv_t]

        def xq_sl(cc, qb):
            return xq3[qb][:, cc, :]

        def xk_sl(cc, kb):
            return xk3[kb][:, cc, :]

        nc.sync.dma_start(out=wq3[:], in_=dram3(wq, 0, CB))
        nc.scalar.dma_start(out=wk3[:], in_=dram3(wk, 0, CB))
        for i in range(4):
            nc.sync.dma_start(out=xq3[i][:], in_=dram3(xqT, 512 * i, 512 * (i + 1)))
            nc.scalar.dma_start(out=xk3[i][:], in_=dram3(xkT, 512 * i, 512 * (i + 1)))
        nc.sync.dma_start(out=wv3[:], in_=dram3(wv, 0, CB))
        for i in range(2):
            nc.sync.dma_start(out=xv3[i][:], in_=dram3(xvT, 1024 * i, 1024 * (i + 1)))
        nc.sync.dma_start(out=wo3[:],
                          in_=wo[:].rearrange("(j p) n -> p j n", p=128))

        nc.vector.memset(v3[:, :, :, D:DV], 1.0)

        # Warm the PE p-state during the input-DMA wait.  One long
        # ACCUMULATION group (start only on the first matmul) so the
        # junk matmuls stream without per-instruction semaphore chains.
        jk = gpool.tile([128, 512], F32, tag="g", name="junk")
        NJUNK = 36
        for i in range(NJUNK):
            nc.tensor.matmul(jk[:], v1_sb[:, 0:128], v1_sb[:, 0:512],
                             start=(i == 0), stop=(i == NJUNK - 1))

        # ---------------- building blocks ----------------
        P_tiles, PV, qk_state = {}, {}, {}

        def scores_pair(h, qb, pair):
            """S^T + exp for chunks (2*pair, 2*pair+1) of head h, qblock qb."""
            st = spool.tile([128, 1024], F32, tag="st", name=f"st_{h}_{qb}_{pair}")
            Pp = P_pool.tile([128, 1024], BF16, tag="P", name=f"P_{h}_{qb}_{pair}")
            for i in range(2):
                kc = 2 * pair + i
                nc.tensor.matmul(st[:, 512 * i:512 * (i + 1)],
                                 k_slice(h, kc), q_slice(h, qb),
                                 start=True, stop=True)
            nc.scalar.activation(Pp[:], st[:], Exp, scale=float(SCALE))
            P_tiles[(h, qb, pair)] = Pp

        def qk_proj_part(w3, x_sl, dst_sb, mb, qb, part, nparts):
            """1/nparts of one [128,512] Q^T/K^T projection group."""
            key = (id(w3), mb, qb)
            if part == 0:
                qk_state[key] = gpool.tile([128, 512], F32, tag="g",
                                           name=f"qk{mb}_{qb}_{id(w3) % 97}")
            ps = qk_state[key]
            step = 8 // nparts
            for cc in range(step * part, step * (part + 1)):
                nc.tensor.matmul(ps[:],
                                 w3[:, cc, 128 * mb:128 * (mb + 1)],
                                 x_sl(cc, qb),
                                 start=(cc == 0), stop=(cc == 7))
            if part == nparts - 1:
                nc.vector.tensor_copy(
                    dst_sb[:, N * mb + 512 * qb:N * mb + 512 * (qb + 1)], ps[:])
                del qk_state[key]

        def v_proj_block(tb):
            """V' for key-chunk tb: out[128 keys, 256] -> v1 cols 0:64."""
            ps = gpool.tile([128, 512], F32, tag="g", name=f"vps{tb}")
            for cc in range(8):
                nc.tensor.matmul(ps[:, 0:CB],
                                 xv3[tb // 8][:, cc, 128 * (tb % 8):
                                              128 * (tb % 8) + 128],
                                 wv3[:, cc, :],
                                 start=(cc == 0), stop=(cc == 7))
            nc.vector.tensor_copy(
                v3[:, tb, :, 0:D],
                ps[:, 0:CB].rearrange("p (h d) -> p h d", d=D))

        def pv_part(h, qb, pair):
            """Two PV chunk-matmuls for head h / qblock qb; finishes at pair 7.

            po rows 0:64 = O^T(h) raw, row 64 = softmax denominator.
            """
            if pair == 0:
                PV[(h, qb)] = pvpool.tile([128, 512], F32, tag="po",
                                          name=f"po{h}_{qb}")
            po = PV[(h, qb)]
            Pp = P_tiles.pop((h, qb, pair))
            for i in range(2):
                kc = 2 * pair + i
                nc.tensor.matmul(po[0:DV, :],
                                 v3[:, kc, h, :],
                                 Pp[:, 512 * i:512 * (i + 1)],
                                 start=(kc == 0), stop=(kc == NCHUNK - 1))
            if pair == 7:
                pv_finish(h, qb)

        def pv_finish(h, qb):
            """Normalize: A^T(h) = po[0:64] / po[64] -> aT plane."""
            po = PV.pop((h, qb))
            dinv = dpool.tile([64, 512], F32, tag="dinv", name=f"di{h}_{qb}")
            nc.vector.tensor_copy(draw_sb[:], po[64:65, :])
            nc.vector.reciprocal_approx_fast(drow_sb[:], draw_sb[:])
            nc.gpsimd.partition_broadcast(dinv[:], drow_sb[:])
            dst = aT0_sb if h < 2 else aT1_sb
            nc.vector.tensor_mul(
                dst[64 * (h % 2):64 * (h % 2) + 64, 512 * qb:512 * (qb + 1)],
                po[0:D, :], dinv[:])

        def oproj_m(qb, m, scalar_cast=False):
            """One m-block of the out-proj partial for query block qb.

            scalar_cast routes the psum->sbuf cast to ScalarE (a Copy,
            resident in every activation table set) -- used after the
            final exp, when ScalarE is idle, to unserialize the tail.
            """
            ps = gpool.tile([128, 512], F32, tag="g", name=f"ops{m}_{qb}")
            for j in range(2):
                aT = (aT0_sb, aT1_sb)[j]
                nc.tensor.matmul(ps[:], wo3[:, j, 128 * m:128 * (m + 1)],
                                 aT[:, 512 * qb:512 * (qb + 1)],
                                 start=(j == 0), stop=(j == 1))
            ev = ospool.tile([128, 512], BF16, tag="ev", name=f"oev{m}_{qb}")
            if scalar_cast:
                nc.scalar.copy(ev[:], ps[:])
            else:
                nc.vector.tensor_copy(ev[:], ps[:])
            nc.sync.dma_start(
                out=outT[128 * m:128 * (m + 1), 512 * qb:512 * (qb + 1)],
                in_=ev[:])

        # ---------------- emission ----------------
        # Pre-loop: Q^T(qb0) both planes + K^T plane-0 blocks 0/1 -- the
        # minimum for the first four score pairs.  K blocks 2/3 ride as
        # slot items ahead of the pairs that need them, so the first
        # exp is gated only by the first xq/xk quarters.
        for mb in range(2):
            qk_proj_part(wq3, xq_sl, qT_sb, mb, 0, 0, 1)
        for kb in range(2):
            qk_proj_part(wk3, xk_sl, kT_sb, 0, kb, 0, 1)

        # Slot schedule: one slot = one score group = one exp (~1.1us);
        # each slot carries <=~1.3us of extra PE work.
        #   qb0 h0: K^T plane-1 half-groups    (needed by h2 scores)
        #       h1: V' chunk 2p
        #       h2: V' chunk 2p+1 (pre), pv(h0) pair p
        #       h3: pv(h1) p, pv(h2) p-1, Q^T(qb1) quarters on p<4
        #       tail: pv(h2) pair 7        [pv(h3) rides in qb1's slots]
        #   qb>=1: pv(h-1) in-slot; fillers from the item list below;
        #       h3 additionally slot-lags pv(h3) by one pair, tail = pair 7.
        def run_qblock(qb, pre_items, post_items):
            lag = 2 if qb == 0 else 1
            for h in range(HC):
                for pair in range(8):
                    scores_pair(h, qb, pair)
                    for it in pre_items.get((h, pair), ()):
                        it()
                    if h >= lag:
                        pv_part(h - lag, qb, pair)
                    if h == HC - 1 and lag == 1 and pair >= 1:
                        pv_part(HC - 1, qb, pair - 1)
                    for it in post_items.get((h, pair), ()):
                        it()
            if qb == 0:
                pv_part(HC - 2, 0, 7)
            else:
                pv_part(HC - 1, qb, 7)

        pre0, post0 = {}, {}
        # K plane-0 blocks 2/3 ride the earliest h0 slots (well before
        # score pairs 4 and 6 need them); the displaced K plane-1
        # halves slide into h1 slots.
        post0[(0, 0)] = [lambda: qk_proj_part(wk3, xk_sl, kT_sb, 0, 2, 0, 1)]
        post0[(0, 1)] = [lambda: qk_proj_part(wk3, xk_sl, kT_sb, 0, 3, 0, 1)]
        km1 = [lambda kb=kb, part=part:
               qk_proj_part(wk3, xk_sl, kT_sb, 1, kb, part, 2)
               for kb in range(NQB) for part in range(2)]
        for p in (2, 3, 4, 5, 6, 7):
            post0[(0, p)] = [km1.pop(0)]
        for p in range(8):
            post0[(1, p)] = ([km1.pop(0)] if km1 else []) + \
                [lambda tb=2 * p: v_proj_block(tb)]
            pre0[(2, p)] = [lambda tb=2 * p + 1: v_proj_block(tb)]
            post0[(3, p)] = [lambda pp=p - 1: pv_part(2, 0, pp)] if p >= 1 else []
            if p < 4:
                post0[(3, p)] = post0.get((3, p), []) + \
                    [lambda mb=p // 2, part=p % 2:
                     qk_proj_part(wq3, xq_sl, qT_sb, mb, 1, part, 2)]
        run_qblock(0, pre0, post0)
        xk_stack.close()
        xv_stack.close()

        for qb in range(1, NQB):
            items = []
            if qb == 1:
                items += [lambda p=p: pv_part(3, 0, p) for p in range(8)]
            opq = [lambda m=m, q=qb - 1: oproj_m(q, m) for m in range(8)]
            if qb < NQB - 1:
                qqs = [lambda mb=mb, part=part, q=qb + 1:
                       qk_proj_part(wq3, xq_sl, qT_sb, mb, q, part, 4)
                       for mb in range(2) for part in range(4)]
                inter = [x for pair in zip(opq, qqs) for x in pair]
            else:
                inter = opq
            items += inter
            sched = {}
            for s, it in enumerate(items):
                sched[(s // 8, s % 8)] = sched.get((s // 8, s % 8), []) + [it]
            run_qblock(qb, {}, sched)
            if qb == NQB - 1:
                # keep the PE clock warm through the normalize chain so
                # the tail out-proj runs at full speed
                jk2 = gpool.tile([128, 512], F32, tag="g", name="junk2")
                for i in range(8):
                    nc.tensor.matmul(jk2[:], v1_sb[:, 0:128], v1_sb[:, 0:512],
                                     start=(i == 0), stop=(i == 7))
            if qb == NQB - 2:
                xq_stack.close()
        pv_finish(HC - 1, NQB - 1) if (HC - 1, NQB - 1) in PV else None
        for m in range(8):
            oproj_m(NQB - 1, m, scalar_cast=(m % 2 == 1))
        main.close()

    nc.compile()
    return nc


def _get_nc():
    if "nc" not in _CACHE:
        _CACHE["nc"] = build_nc()
    return _CACHE["nc"]


def _make_in_maps(q, k, v, Wq, Wk, Wv, Wo):
    bf = ml_dtypes.bfloat16
    q, k, v = np.asarray(q), np.asarray(k), np.asarray(v)
    qT = [np.ascontiguousarray(q[b].T).astype(bf) for b in range(B)]
    kT = [np.ascontiguousarray(k[b].T).astype(bf) for b in range(B)]
    vT = [np.ascontiguousarray(v[b].T).astype(bf) for b in range(B)]
    Wq, Wk, Wv, Wo = (np.asarray(x) for x in (Wq, Wk, Wv, Wo))
    wq_s = [np.ascontiguousarray(Wq[:, CB * g:CB * (g + 1)]).astype(bf)
            for g in range(4)]
    wk_s = [np.ascontiguousarray(Wk[:, CB * g:CB * (g + 1)]).astype(bf)
            for g in range(4)]
    wv_s = [np.ascontiguousarray(Wv[:, CB * g:CB * (g + 1)]).astype(bf)
            for g in range(4)]
    wo_s = [np.ascontiguousarray(Wo[CB * g:CB * (g + 1), :]).astype(bf)
            for g in range(4)]
    in_maps = []
    for c in range(8):
        b, g = c // 4, c % 4
        in_maps.append({
            "xqT": qT[b], "xkT": kT[b], "xvT": vT[b],
            "wq": wq_s[g], "wk": wk_s[g], "wv": wv_s[g], "wo": wo_s[g],
        })
    return in_maps


def _run(inputs, trace=False, **kw):
    nc = _get_nc()
    in_maps = _make_in_maps(inputs["q"], inputs["k"], inputs["v"],
                            inputs["Wq"], inputs["Wk"], inputs["Wv"], inputs["Wo"])
    res = None
    for attempt in range(3):
        try:
            res = run_bass_kernel_spmd(nc, in_maps, core_ids=list(range(8)),
                                       trace=trace, **kw)
            break
        except Exception:
            if attempt == 2:
                raise
            import time
            time.sleep(2.0)
    out = np.empty((B, N, C), np.float32)
    for b in range(B):
        acc = np.zeros((C, N), np.float32)
        for g in range(4):
            acc += res.results[4 * b + g]["outT"].astype(np.float32)
        out[b] = acc.T
    return out, res


def kernel(**inputs) -> np.ndarray:
    out, _ = _run(inputs, trace=False)
    return out



# revision 4
# speedup vs baseline: 1.0806x; 1.0806x over previous
"""Distributed multi-head attention kernel for 8 TRN2 NeuronCores.

Problem: B=2, N=2048, C=1024, H=16 heads, D=64.
  out = softmax((q@Wq)(k@Wk)^T / sqrt(D)) @ (v@Wv) @ Wo   (per head, biases zero)

Sharding: batch x head-group.  Core c owns batch b=c//4 and head group
g=c%4 -> heads [4g, 4g+4) = channel block [256g, 256g+256).
Zero-redundancy: each core projects only its own 256 Q/K/V channels for
its batch, runs attention for its 4 heads over all 2048 queries/keys,
and computes the row-sharded out-proj partial out^T = Wo_s^T @ A^T
(bf16).  The host sums the 4 partials per batch (the "all-reduce" of
the sharding hint, done at gather time) -- no device collectives.

v2 over the 244-247us baseline:
  - Score matmuls contract over only D=64, so the two heads of a plane
    (rows 0:64 / 64:128) run CONCURRENTLY in distinct PE row-groups
    (tile_position auto-derives from base_partition).  Each score tile
    is [128 keys, h_even 512 | h_odd 512] for one key chunk; the pair
    of matmuls overlaps ~2x, halving score PE wall time (~27us).
  - Inputs land as fully contiguous per-partition runs (8KB+) -- the
    host pre-shuffles DRAM layouts -- so each queue streams near peak
    instead of the ~170GB/s 1KB-packet rate, and trigger cost drops.
  - Slot = (head-pair hp, query-block qb, kc-pair): 4 score mms +
    2 exps (~2.29us ScalarE).  PV of the lagging group (one hp-group
    lag), projections and out-proj ride inside slots; steady state is
    ScalarE-bound at ~2.3us/slot x 128 slots.
"""

import sys

sys.path.insert(0, "/opt/trn_rl_repo")

from contextlib import ExitStack

import numpy as np
import ml_dtypes

import concourse.bass as bass
import concourse.bacc as bacc
import concourse.mybir as mybir
import concourse.tile as tile
from concourse.bass_utils import run_bass_kernel_spmd

BF16 = mybir.dt.bfloat16
F32 = mybir.dt.float32
Exp = mybir.ActivationFunctionType.Exp

B, N, C = 2, 2048, 1024
H, D = 16, 64
HC = 4              # heads per core
CB = HC * D         # own channel block = 256
DV = D + 1          # V cols per head incl. ones column
NCHUNK = N // 128   # 16 key chunks
NQB = N // 512      # 4 query blocks
SCALE = 1.0 / np.sqrt(D)

_CACHE = {}


def build_nc():
    nc = bacc.Bacc("TRN2", target_bir_lowering=False, debug=False, num_devices=8)

    # DRAM layouts are pre-shuffled host-side so every transfer is a
    # contiguous >=8KB per-partition run (see _make_in_maps).
    xq = nc.declare_dram_parameter("xq", [128, 4 * 4096], BF16, isOutput=False)
    xk = nc.declare_dram_parameter("xk", [128, 4 * 4096], BF16, isOutput=False)
    xv = nc.declare_dram_parameter("xv", [128, 4 * 4096], BF16, isOutput=False)
    wq = nc.declare_dram_parameter("wq", [128, 2048], BF16, isOutput=False)
    wk = nc.declare_dram_parameter("wk", [128, 2048], BF16, isOutput=False)
    wv = nc.declare_dram_parameter("wv", [128, 2048], BF16, isOutput=False)
    wo = nc.declare_dram_parameter("wo", [128, 2048], BF16, isOutput=False)
    outT = nc.declare_dram_parameter("outT", [C, N], BF16, isOutput=True)

    with tile.TileContext(nc) as tc, ExitStack() as top:
        # ---------------- resident SBUF ----------------
        res = top.enter_context(tc.tile_pool(name="res", bufs=1))
        # Q^T / K^T: plane p holds head 2p in rows 0:64, head 2p+1 in 64:128
        qT_sb = res.tile([128, 2 * N], BF16, tag="qT")
        kT_sb = res.tile([128, 2 * N], BF16, tag="kT")
        # V' is 65 cols per (kc, h): col 64 = ones so the PV matmul
        # accumulates the softmax denominator in psum row 64.
        v1_sb = res.tile([128, NCHUNK * HC * DV], BF16, tag="v1")
        aT0_sb = res.tile([128, N], BF16, tag="aT0")   # A^T rows 0:128 (h 0,1)
        aT1_sb = res.tile([128, N], BF16, tag="aT1")   # A^T rows 128:256 (h 2,3)
        draw_sb = res.tile([1, 512], F32, tag="draw")
        drow_sb = res.tile([1, 512], F32, tag="drow")

        wq_t = res.tile([128, 2048], BF16, tag="wqt")
        wk_t = res.tile([128, 2048], BF16, tag="wkt")
        wv_t = res.tile([128, 2048], BF16, tag="wvt")
        wo_t = res.tile([128, 2048], BF16, tag="wot")
        wq3 = wq_t[:].rearrange("p (c n) -> p c n", c=8)
        wk3 = wk_t[:].rearrange("p (c n) -> p c n", c=8)
        wv3 = wv_t[:].rearrange("p (c n) -> p c n", c=8)
        wo3 = wo_t[:].rearrange("p (j n) -> p j n", j=2)

        def q_slice(h, qb):
            base = N * (h // 2)
            return qT_sb[64 * (h % 2):64 * (h % 2) + 64,
                         base + 512 * qb:base + 512 * (qb + 1)]

        def k_slice(h, kc):
            base = N * (h // 2)
            return kT_sb[64 * (h % 2):64 * (h % 2) + 64,
                         base + 128 * kc:base + 128 * (kc + 1)]

        v3 = v1_sb[:].rearrange("p (kc h x) -> p kc h x", kc=NCHUNK, x=DV)

        # ---------------- pools ----------------
        main = ExitStack()
        P_pool = main.enter_context(tc.tile_pool(name="P_pool", bufs=25))
        dpool = main.enter_context(tc.tile_pool(name="dpool", bufs=2))
        ospool = main.enter_context(tc.tile_pool(name="ospool", bufs=3))
        spool = main.enter_context(
            tc.tile_pool(name="spool", bufs=2, space="PSUM"))   # 2x2 banks
        pvpool = main.enter_context(
            tc.tile_pool(name="pvpool", bufs=2, space="PSUM"))  # 2x1 banks
        gpool = main.enter_context(
            tc.tile_pool(name="gpool", bufs=2, space="PSUM"))   # 2x1 banks
        xq_stack = ExitStack()
        xqpool = xq_stack.enter_context(
            tc.tile_pool(name="xqpool", bufs=4, side="right"))
        xv_stack = ExitStack()
        xvpool = xv_stack.enter_context(
            tc.tile_pool(name="xvpool", bufs=2, side="right"))
        xk_stack = ExitStack()
        xkpool = xk_stack.enter_context(
            tc.tile_pool(name="xkpool", bufs=4, side="right"))

        xq_t = [xqpool.tile([128, 4096], BF16, tag="xq", name=f"xq{i}")
                for i in range(4)]
        xk_t = [xkpool.tile([128, 4096], BF16, tag="xk", name=f"xk{i}")
                for i in range(4)]
        xv_t = [xvpool.tile([128, 8192], BF16, tag="xv", name=f"xv{i}")
                for i in range(2)]
        xq3 = [t[:].rearrange("p (c n) -> p c n", c=8) for t in xq_t]
        xk3 = [t[:].rearrange("p (c n) -> p c n", c=8) for t in xk_t]
        xv3 = [t[:].rearrange("p (c n) -> p c n", c=8) for t in xv_t]

        def xq_sl(cc, qb):
            return xq3[qb][:, cc, :]

        def xk_sl(cc, kb):
            return xk3[kb][:, cc, :]

        # -------- input DMA: contiguous runs, need-ordered queues --------
        # sync queue: q-side + wo; scalar queue: k-side + v-side.
        nc.sync.dma_start(out=wq_t[:], in_=wq[:])
        for i in range(4):
            nc.sync.dma_start(out=xq_t[i][:], in_=xq[:, 4096 * i:4096 * (i + 1)])
        nc.sync.dma_start(out=wo_t[:], in_=wo[:])
        nc.scalar.dma_start(out=wk_t[:], in_=wk[:])
        nc.scalar.dma_start(out=xk_t[0][:], in_=xk[:, 0:4096])
        nc.scalar.dma_start(out=xk_t[1][:], in_=xk[:, 4096:8192])
        nc.scalar.dma_start(out=xk_t[2][:], in_=xk[:, 8192:12288])
        nc.scalar.dma_start(out=wv_t[:], in_=wv[:])
        nc.scalar.dma_start(out=xk_t[3][:], in_=xk[:, 12288:16384])
        for j in range(2):
            for q in range(2):
                nc.scalar.dma_start(
                    out=xv_t[j][:, 4096 * q:4096 * (q + 1)],
                    in_=xv[:, 8192 * j + 4096 * q:8192 * j + 4096 * (q + 1)])

        nc.vector.memset(v3[:, :, :, D:DV], 1.0)

        # Warm the PE p-state during the input-DMA wait.  One long
        # ACCUMULATION group (start only on the first matmul) so the
        # junk matmuls stream without per-matmul semaphores.
        jk = gpool.tile([128, 512], F32, tag="g", name="junk")
        NJUNK = 36
        for i in range(NJUNK):
            nc.tensor.matmul(jk[:], v1_sb[:, 0:128], v1_sb[:, 0:512],
                             start=(i == 0), stop=(i == NJUNK - 1))

        # ---------------- building blocks ----------------
        P_tiles, PV, qk_state = {}, {}, {}

        def scores_kc(hp, qb, kc):
            """One [128, 1024] score tile: kc for heads (2hp | 2hp+1).

            The two matmuls target PE row-groups (0,0) and (64,0) and
            run concurrently.
            """
            st = spool.tile([128, 1024], F32, tag="st", name=f"st{hp}_{qb}_{kc}")
            nc.tensor.matmul(st[:, 0:512], k_slice(2 * hp, kc),
                             q_slice(2 * hp, qb), start=True, stop=True)
            nc.tensor.matmul(st[:, 512:1024], k_slice(2 * hp + 1, kc),
                             q_slice(2 * hp + 1, qb), start=True, stop=True)
            Pp = P_pool.tile([128, 1024], BF16, tag="P", name=f"P{hp}_{qb}_{kc}")
            nc.scalar.activation(Pp[:], st[:], Exp, scale=float(SCALE))
            P_tiles[(hp, qb, kc)] = Pp

        def qk_proj(w3, x_sl, dst_sb, mb, qb):
            """One [128,512] Q^T/K^T projection group (8 cc matmuls)."""
            ps = gpool.tile([128, 512], F32, tag="g",
                            name=f"qk{mb}_{qb}_{id(w3) % 97}")
            for cc in range(8):
                nc.tensor.matmul(ps[:],
                                 w3[:, cc, 128 * mb:128 * (mb + 1)],
                                 x_sl(cc, qb),
                                 start=(cc == 0), stop=(cc == 7))
            nc.vector.tensor_copy(
                dst_sb[:, N * mb + 512 * qb:N * mb + 512 * (qb + 1)], ps[:])

        def v_proj_block(tb):
            """V' for key-chunk tb: out[128 keys, 256] -> v1 cols 0:64."""
            ps = gpool.tile([128, 512], F32, tag="g", name=f"vps{tb}")
            for cc in range(8):
                nc.tensor.matmul(ps[:, 0:CB],
                                 xv3[tb // 8][:, cc, 128 * (tb % 8):
                                              128 * (tb % 8) + 128],
                                 wv3[:, cc, :],
                                 start=(cc == 0), stop=(cc == 7))
            nc.vector.tensor_copy(
                v3[:, tb, :, 0:D],
                ps[:, 0:CB].rearrange("p (h d) -> p h d", d=D))

        def pv_kc(hp, qb, kc):
            """PV chunk kc for both heads of hp; finishes at kc 15."""
            Pp = P_tiles[(hp, qb, kc)]
            for j in range(2):
                h = 2 * hp + j
                if kc == 0:
                    PV[(h, qb)] = pvpool.tile([128, 512], F32, tag="po",
                                              name=f"po{h}_{qb}")
                po = PV[(h, qb)]
                nc.tensor.matmul(po[0:DV, :],
                                 v3[:, kc, h, :],
                                 Pp[:, 512 * j:512 * (j + 1)],
                                 start=(kc == 0), stop=(kc == NCHUNK - 1))
            P_tiles.pop((hp, qb, kc))
            if kc == NCHUNK - 1:
                pv_finish(2 * hp, qb)
                pv_finish(2 * hp + 1, qb)

        def pv_finish(h, qb):
            """Normalize: A^T(h) = po[0:64] / po[64] -> aT plane."""
            po = PV.pop((h, qb))
            dinv = dpool.tile([64, 512], F32, tag="dinv", name=f"di{h}_{qb}")
            nc.vector.tensor_copy(draw_sb[:], po[64:65, :])
            nc.vector.reciprocal_approx_fast(drow_sb[:], draw_sb[:])
            nc.gpsimd.partition_broadcast(dinv[:], drow_sb[:])
            dst = aT0_sb if h < 2 else aT1_sb
            nc.vector.tensor_mul(
                dst[64 * (h % 2):64 * (h % 2) + 64, 512 * qb:512 * (qb + 1)],
                po[0:D, :], dinv[:])

        def oproj_m(qb, m, scalar_cast=False):
            """One m-block of the out-proj partial for query block qb."""
            ps = gpool.tile([128, 512], F32, tag="g", name=f"ops{m}_{qb}")
            for j in range(2):
                aT = (aT0_sb, aT1_sb)[j]
                nc.tensor.matmul(ps[:], wo3[:, j, 128 * m:128 * (m + 1)],
                                 aT[:, 512 * qb:512 * (qb + 1)],
                                 start=(j == 0), stop=(j == 1))
            ev = ospool.tile([128, 512], BF16, tag="ev", name=f"oev{m}_{qb}")
            if scalar_cast:
                nc.scalar.copy(ev[:], ps[:])
            else:
                nc.vector.tensor_copy(ev[:], ps[:])
            nc.sync.dma_start(
                out=outT[128 * m:128 * (m + 1), 512 * qb:512 * (qb + 1)],
                in_=ev[:])

        # ---------------- emission ----------------
        # Slot (hp, qb, pair): scores+exp for kc 2*pair, 2*pair+1 of the
        # head pair, with rider work interleaved between the two kc's
        # (mid) and after (post).  PV of group g rides group g+1's slots.
        def run_group(hp, qb, mid_items, post_items, pv_src=None):
            for pair in range(8):
                scores_kc(hp, qb, 2 * pair)
                for it in mid_items.get(pair, ()):
                    it()
                scores_kc(hp, qb, 2 * pair + 1)
                if pv_src is not None:
                    pv_kc(pv_src[0], pv_src[1], 2 * pair)
                    pv_kc(pv_src[0], pv_src[1], 2 * pair + 1)
                for it in post_items.get(pair, ()):
                    it()

        def run_final_group():
            """(qb3, hp1): pv of (hp0, qb3) front-loaded on pairs 0-3 so
            its two PSUM tiles free up for (hp1, qb3)'s pv on pairs 4-7;
            all PV and the last exp finish in-slot, tail is just
            normalize + out-proj."""
            qb = NQB - 1
            for pair in range(8):
                hp_src, base = (0, 4 * pair) if pair < 4 else (1, 4 * (pair - 4))
                scores_kc(1, qb, 2 * pair)
                pv_kc(hp_src, qb, base)
                pv_kc(hp_src, qb, base + 1)
                scores_kc(1, qb, 2 * pair + 1)
                pv_kc(hp_src, qb, base + 2)
                pv_kc(hp_src, qb, base + 3)
                oproj_m(qb - 1, pair)

        # Pre-loop: only what the first slot needs.
        qk_proj(wq3, xq_sl, qT_sb, 0, 0)
        qk_proj(wk3, xk_sl, kT_sb, 0, 0)

        # ---- group (qb0, hp0): K/Q/V projections ride ----
        mid0 = {
            0: [lambda: qk_proj(wk3, xk_sl, kT_sb, 0, 1)],
            1: [lambda: qk_proj(wk3, xk_sl, kT_sb, 1, 0)],
            2: [lambda: qk_proj(wk3, xk_sl, kT_sb, 0, 2)],
            3: [lambda: qk_proj(wk3, xk_sl, kT_sb, 1, 1)],
            4: [lambda: qk_proj(wk3, xk_sl, kT_sb, 0, 3)],
            5: [lambda: qk_proj(wk3, xk_sl, kT_sb, 1, 2)],
            6: [lambda: qk_proj(wq3, xq_sl, qT_sb, 1, 0)],
            7: [lambda: qk_proj(wk3, xk_sl, kT_sb, 1, 3)],
        }
        post0 = {
            6: [lambda: v_proj_block(0), lambda: v_proj_block(1)],
            7: [lambda: v_proj_block(2), lambda: v_proj_block(3)],
        }
        run_group(0, 0, mid0, post0)
        xk_stack.close()

        # ---- group (qb0, hp1): pv(qb0,hp0) + V' chunks + Q(qb1) ----
        mid1 = {p: [lambda tb=2 * p + 4: v_proj_block(tb)] for p in range(6)}
        post1 = {p: [lambda tb=2 * p + 5: v_proj_block(tb)] for p in range(6)}
        for p, (mb, q) in zip((2, 3, 4, 5),
                              ((0, 1), (1, 1), (0, 2), (1, 2))):
            post1.setdefault(p, []).append(
                lambda mb=mb, q=q: qk_proj(wq3, xq_sl, qT_sb, mb, q))
        run_group(1, 0, mid1, post1, pv_src=(0, 0))
        xv_stack.close()

        # ---- steady state: qb 1..3 ----
        for qb in range(1, NQB):
            # hp0: pv of (qb-1, hp1); Q proj of qb+1 rides.
            mid, post = {}, {}
            if qb < NQB - 1:
                qq = [(0, qb + 1), (1, qb + 1)]
                post[1] = [lambda mb=qq[0][0], q=qq[0][1]:
                           qk_proj(wq3, xq_sl, qT_sb, mb, q)]
                post[4] = [lambda mb=qq[1][0], q=qq[1][1]:
                           qk_proj(wq3, xq_sl, qT_sb, mb, q)]
            run_group(0, qb, mid, post, pv_src=(1, qb - 1))
            if qb == NQB - 1:
                xq_stack.close()
                run_final_group()
            else:
                # hp1: pv of (qb, hp0); out-proj of qb-1 rides.
                mid, post = {}, {}
                for p in range(8):
                    post[p] = [lambda m=p, q=qb - 1: oproj_m(q, m)]
                run_group(1, qb, mid, post, pv_src=(0, qb))

        # ---- tail: normalize finished in final group; out-proj of qb3 ----
        jk2 = gpool.tile([128, 512], F32, tag="g", name="junk2")
        for i in range(8):
            nc.tensor.matmul(jk2[:], v1_sb[:, 0:128], v1_sb[:, 0:512],
                             start=(i == 0), stop=(i == 7))
        for m in range(8):
            oproj_m(NQB - 1, m, scalar_cast=(m % 2 == 1))
        main.close()

    nc.compile()
    return nc


def _get_nc():
    if "nc" not in _CACHE:
        _CACHE["nc"] = build_nc()
    return _CACHE["nc"]


def _make_in_maps(q, k, v, Wq, Wk, Wv, Wo):
    bf = ml_dtypes.bfloat16
    q, k, v = np.asarray(q), np.asarray(k), np.asarray(v)
    Wq, Wk, Wv, Wo = (np.asarray(x) for x in (Wq, Wk, Wv, Wo))

    # x pre-shuffles: per-core SBUF wants partition p, cc-chunk c, col n
    # contiguous per transfer block.
    def xq_pre(xT):      # [C, N] -> [128, 4*4096], quarters of 512 tokens
        return np.ascontiguousarray(
            xT.reshape(8, 128, 4, 512).transpose(1, 2, 0, 3).reshape(128, -1)
        ).astype(bf)

    def xv_pre(xT):      # [C, N] -> [128, 2*8192], halves of 1024 keys
        return np.ascontiguousarray(
            xT.reshape(8, 128, 2, 1024).transpose(1, 2, 0, 3).reshape(128, -1)
        ).astype(bf)

    def w_pre(W):        # [1024, 256] -> [128, 2048]
        return np.ascontiguousarray(
            W.reshape(8, 128, 256).transpose(1, 0, 2).reshape(128, -1)
        ).astype(bf)

    def wo_pre(W):       # [256, 1024] -> [128, 2048]
        return np.ascontiguousarray(
            W.reshape(2, 128, 1024).transpose(1, 0, 2).reshape(128, -1)
        ).astype(bf)

    qT = [q[b].T for b in range(B)]
    kT = [k[b].T for b in range(B)]
    vT = [v[b].T for b in range(B)]
    in_maps = []
    for c in range(8):
        b, g = c // 4, c % 4
        cs = slice(CB * g, CB * (g + 1))
        in_maps.append({
            "xq": xq_pre(qT[b]), "xk": xq_pre(kT[b]), "xv": xv_pre(vT[b]),
            "wq": w_pre(Wq[:, cs]), "wk": w_pre(Wk[:, cs]),
            "wv": w_pre(Wv[:, cs]), "wo": wo_pre(Wo[cs, :]),
        })
    return in_maps


def _run(inputs, trace=False, **kw):
    nc = _get_nc()
    in_maps = _make_in_maps(inputs["q"], inputs["k"], inputs["v"],
                            inputs["Wq"], inputs["Wk"], inputs["Wv"], inputs["Wo"])
    res = None
    for attempt in range(3):
        try:
            res = run_bass_kernel_spmd(nc, in_maps, core_ids=list(range(8)),
                                       trace=trace, **kw)
            break
        except Exception:
            if attempt == 2:
                raise
            import time
            time.sleep(2.0)
    out = np.empty((B, N, C), np.float32)
    for b in range(B):
        acc = np.zeros((C, N), np.float32)
        for g in range(4):
            acc += res.results[4 * b + g]["outT"].astype(np.float32)
        out[b] = acc.T
    return out, res


def kernel(**inputs) -> np.ndarray:
    out, _ = _run(inputs, trace=False)
    return out


# revision 38
# speedup vs baseline: 1.2042x; 1.1144x over previous
"""Distributed multi-head attention kernel for 8 TRN2 NeuronCores.

Problem: B=2, N=2048, C=1024, H=16 heads, D=64.
  out = softmax((q@Wq)(k@Wk)^T / sqrt(D)) @ (v@Wv) @ Wo   (per head, biases zero)

Sharding: batch x head-group.  Core c owns batch b=c//4 and head group
g=c%4 -> heads [4g, 4g+4) = channel block [256g, 256g+256).
Zero-redundancy: each core projects only its own 256 Q/K/V channels for
its batch, runs attention for its 4 heads over all 2048 queries/keys,
and computes the row-sharded out-proj partial out^T = Wo_s^T @ A^T
(bf16).  The host sums the 4 partials per batch (the "all-reduce" of
the sharding hint, done at gather time) -- no device collectives.

Design (measured ~206us vs the 244.6us single-head-slot baseline):
  - Concurrent score pairs: score matmuls contract over only D=64, so
    the two heads of a plane (PE row-groups (0,0)/(64,0), derived from
    base_partition) run CONCURRENTLY -- each [128 keys, h_even 512 |
    h_odd 512] score tile costs ~0.32us instead of 0.64 (saves ~25us).
  - Slot = (head-pair hp, qb, kc): 2 score tiles + 2 exps = 2.29us of
    ScalarE; steady state is exp-bound (128 x [128,1024] ACTIVATE =
    147us floor; PSUM's 8 banks cannot fit wider exp tiles).  PV of
    the previous hp-group, split projection halves/quarters and
    out-proj ride inside slots, all sized <=1.7us so the next slot's
    scores are never far behind the in-order PE queue.
  - Inputs stream as contiguous >=8KB per-partition runs (host
    pre-shuffles DRAM layouts) in GLOBAL need-order across the two
    HWDGE queues + the gpsimd SWDGE queue (aggregate ~290GB/s; each
    queue alone tops at ~150).  The Tile scheduler reorders same-queue
    DMA triggers, so explicit scheduling deps chain each queue and the
    pool queue is gated on xk0 to protect the critical first MBs.
    First exp fires at ~23us (DMA-bound).
  - PV is V'-stationary: V' col 64 = ones accumulates the softmax
    denominator in psum row 64.  Normalize stages the raw numerators
    to SBUF first so the PV psum bank frees in ~0.9us, then
    reciprocal + gpsimd partition-broadcast + multiply into the A^T
    plane (group-boundary stall eliminated).
  - Final group front-loads pv(hp0,qb3) on pairs 0-3 (its two psum
    banks then serve pv(hp1,qb3) on pairs 4-7), so only normalize +
    out-proj remain after the last exp; late out-proj casts/DMAs use
    ScalarE + both HW queues; junk-matmul groups bridge HAM clock-
    warmup at start and tail.
"""

import sys

sys.path.insert(0, "/opt/trn_rl_repo")

from contextlib import ExitStack

import numpy as np
import ml_dtypes

import concourse.bass as bass
import concourse.bacc as bacc
import concourse.mybir as mybir
import concourse.tile as tile
from concourse.bass_utils import run_bass_kernel_spmd

BF16 = mybir.dt.bfloat16
F32 = mybir.dt.float32
Exp = mybir.ActivationFunctionType.Exp

B, N, C = 2, 2048, 1024
H, D = 16, 64
HC = 4              # heads per core
CB = HC * D         # own channel block = 256
DV = D + 1          # V cols per head incl. ones column
NCHUNK = N // 128   # 16 key chunks
NQB = N // 512      # 4 query blocks
SCALE = 1.0 / np.sqrt(D)

_CACHE = {}


def build_nc():
    nc = bacc.Bacc("TRN2", target_bir_lowering=False, debug=False, num_devices=8)

    # DRAM layouts are pre-shuffled host-side so every transfer is a
    # contiguous >=8KB per-partition run (see _make_in_maps).
    xq = nc.declare_dram_parameter("xq", [128, 4 * 4096], BF16, isOutput=False)
    xk = nc.declare_dram_parameter("xk", [128, 4 * 4096], BF16, isOutput=False)
    xv = nc.declare_dram_parameter("xv", [128, 4 * 4096], BF16, isOutput=False)
    wq = nc.declare_dram_parameter("wq", [128, 2048], BF16, isOutput=False)
    wk = nc.declare_dram_parameter("wk", [128, 2048], BF16, isOutput=False)
    wv = nc.declare_dram_parameter("wv", [128, 2048], BF16, isOutput=False)
    wo = nc.declare_dram_parameter("wo", [128, 2048], BF16, isOutput=False)
    outT = nc.declare_dram_parameter("outT", [C, N], BF16, isOutput=True)

    with tile.TileContext(nc) as tc, ExitStack() as top:
        # ---------------- resident SBUF ----------------
        res = top.enter_context(tc.tile_pool(name="res", bufs=1))
        # Q^T / K^T: plane p holds head 2p in rows 0:64, head 2p+1 in 64:128
        qT_sb = res.tile([128, 2 * N], BF16, tag="qT")
        kT_sb = res.tile([128, 2 * N], BF16, tag="kT")
        # V' is 65 cols per (kc, h): col 64 = ones so the PV matmul
        # accumulates the softmax denominator in psum row 64.
        v1_sb = res.tile([128, NCHUNK * HC * DV], BF16, tag="v1")
        aT0_sb = res.tile([128, N], BF16, tag="aT0")   # A^T rows 0:128 (h 0,1)
        aT1_sb = res.tile([128, N], BF16, tag="aT1")   # A^T rows 128:256 (h 2,3)
        draw_sb = res.tile([1, 512], F32, tag="draw")
        drow_sb = res.tile([1, 512], F32, tag="drow")

        wq_t = res.tile([128, 2048], BF16, tag="wqt")
        wk_t = res.tile([128, 2048], BF16, tag="wkt")
        wv_t = res.tile([128, 2048], BF16, tag="wvt")
        wo_t = res.tile([128, 2048], BF16, tag="wot")
        wq3 = wq_t[:].rearrange("p (c n) -> p c n", c=8)
        wk3 = wk_t[:].rearrange("p (c n) -> p c n", c=8)
        wv3 = wv_t[:].rearrange("p (c n) -> p c n", c=8)
        wo3 = wo_t[:].rearrange("p (j n) -> p j n", j=2)

        def q_slice(h, qb):
            base = N * (h // 2)
            return qT_sb[64 * (h % 2):64 * (h % 2) + 64,
                         base + 512 * qb:base + 512 * (qb + 1)]

        def k_slice(h, kc):
            base = N * (h // 2)
            return kT_sb[64 * (h % 2):64 * (h % 2) + 64,
                         base + 128 * kc:base + 128 * (kc + 1)]

        v3 = v1_sb[:].rearrange("p (kc h x) -> p kc h x", kc=NCHUNK, x=DV)

        # ---------------- pools ----------------
        main = ExitStack()
        P_pool = main.enter_context(tc.tile_pool(name="P_pool", bufs=20))
        dpool = main.enter_context(tc.tile_pool(name="dpool", bufs=4))
        ospool = main.enter_context(tc.tile_pool(name="ospool", bufs=3))
        spool = main.enter_context(
            tc.tile_pool(name="spool", bufs=2, space="PSUM"))   # 2x2 banks
        pvpool = main.enter_context(
            tc.tile_pool(name="pvpool", bufs=2, space="PSUM"))  # 2x1 banks
        gpool = main.enter_context(
            tc.tile_pool(name="gpool", bufs=2, space="PSUM"))   # 2x1 banks
        xq_stack = ExitStack()
        xqpool = xq_stack.enter_context(
            tc.tile_pool(name="xqpool", bufs=4, side="right"))
        xv_stack = ExitStack()
        xvpool = xv_stack.enter_context(
            tc.tile_pool(name="xvpool", bufs=2, side="right"))
        xk_stack = ExitStack()
        xkpool = xk_stack.enter_context(
            tc.tile_pool(name="xkpool", bufs=4, side="right"))

        xq_t = [xqpool.tile([128, 4096], BF16, tag="xq", name=f"xq{i}")
                for i in range(4)]
        xk_t = [xkpool.tile([128, 4096], BF16, tag="xk", name=f"xk{i}")
                for i in range(4)]
        xv_t = [xvpool.tile([128, 8192], BF16, tag="xv", name=f"xv{i}")
                for i in range(2)]
        xq3 = [t[:].rearrange("p (c n) -> p c n", c=8) for t in xq_t]
        xk3 = [t[:].rearrange("p (c n) -> p c n", c=8) for t in xk_t]
        xv3 = [t[:].rearrange("p (c n) -> p c n", c=8) for t in xv_t]

        def xq_sl(cc, qb):
            return xq3[qb][:, cc, :]

        def xk_sl(cc, kb):
            return xk3[kb][:, cc, :]

        # -------- input DMA: contiguous runs over three queues --------
        # All queues share the 16 SDMA engines (~290-350GB/s aggregate),
        # so what matters is GLOBAL need-order.  The Tile scheduler
        # reorders same-queue DMA triggers, so chain each queue's
        # transfers with explicit scheduling deps; the pool (SWDGE)
        # queue is gated on xk0 so it cannot steal engine slots from
        # the critical first megabytes.
        def chain(prev, inst, sync=False):
            if prev is not None:
                tile.add_dep_helper(inst.ins, prev.ins, sync=sync,
                                    reason="dma need-order")
            return inst

        def xv_q(q):
            j, qq = q // 2, q % 2
            return dict(
                out=xv_t[j][:, 4096 * qq:4096 * (qq + 1)],
                in_=xv[:, 4096 * q:4096 * (q + 1)])

        p = None
        p = chain(p, nc.sync.dma_start(out=wq_t[:], in_=wq[:]))
        p = chain(p, nc.sync.dma_start(out=xq_t[0][:], in_=xq[:, 0:4096]))
        p = chain(p, nc.sync.dma_start(out=xk_t[1][:], in_=xk[:, 4096:8192]))
        p = chain(p, nc.sync.dma_start(**xv_q(2)))
        for i in range(1, 4):
            p = chain(p, nc.sync.dma_start(
                out=xq_t[i][:], in_=xq[:, 4096 * i:4096 * (i + 1)]))
        p = None
        p = chain(p, nc.scalar.dma_start(out=wk_t[:], in_=wk[:]))
        xk0_dma = p = chain(p, nc.scalar.dma_start(
            out=xk_t[0][:], in_=xk[:, 0:4096]))
        p = chain(p, nc.scalar.dma_start(out=xk_t[2][:], in_=xk[:, 8192:12288]))
        p = chain(p, nc.scalar.dma_start(out=wv_t[:], in_=wv[:]))
        p = chain(p, nc.scalar.dma_start(**xv_q(1)))
        p = chain(p, nc.scalar.dma_start(out=wo_t[:], in_=wo[:]))
        p = None
        p = chain(p, nc.gpsimd.dma_start(
            out=xk_t[3][:], in_=xk[:, 12288:16384]))
        tile.add_dep_helper(p.ins, xk0_dma.ins, sync=True, reason="pool gate")
        p = chain(p, nc.gpsimd.dma_start(**xv_q(0)))
        p = chain(p, nc.gpsimd.dma_start(**xv_q(3)))

        nc.vector.memset(v3[:, :, :, D:DV], 1.0)

        # Warm the PE p-state during the input-DMA wait.  One long
        # ACCUMULATION group (start only on the first matmul) so the
        # junk matmuls stream without per-matmul semaphores.  Reads
        # aT0 (written only ~40us later) so it has NO input deps and
        # starts the moment the PE comes up.
        jk = spool.tile([128, 1024], F32, tag="st", name="junk")
        NJUNK = 40
        for i in range(NJUNK):
            nc.tensor.matmul(jk[:, 0:512], aT0_sb[:, 0:128], aT0_sb[:, 0:512],
                             start=(i == 0), stop=(i == NJUNK - 1))

        # ---------------- building blocks ----------------
        P_tiles, PV, qk_state = {}, {}, {}

        def scores_kc(hp, qb, kc):
            """One [128, 1024] score tile: kc for heads (2hp | 2hp+1).

            The two matmuls target PE row-groups (0,0) and (64,0) and
            run concurrently.
            """
            st = spool.tile([128, 1024], F32, tag="st", name=f"st{hp}_{qb}_{kc}")
            nc.tensor.matmul(st[:, 0:512], k_slice(2 * hp, kc),
                             q_slice(2 * hp, qb), start=True, stop=True)
            nc.tensor.matmul(st[:, 512:1024], k_slice(2 * hp + 1, kc),
                             q_slice(2 * hp + 1, qb), start=True, stop=True)
            Pp = P_pool.tile([128, 1024], BF16, tag="P", name=f"P{hp}_{qb}_{kc}")
            nc.scalar.activation(Pp[:], st[:], Exp, scale=float(SCALE))
            P_tiles[(hp, qb, kc)] = Pp

        def qk_proj_part(w3, x_sl, dst_sb, mb, qb, part, nparts):
            """1/nparts of one [128,512] Q^T/K^T projection group."""
            key = (id(w3), mb, qb)
            if part == 0:
                qk_state[key] = gpool.tile([128, 512], F32, tag="g",
                                           name=f"qk{mb}_{qb}_{id(w3) % 97}")
            ps = qk_state[key]
            step = 8 // nparts
            for cc in range(step * part, step * (part + 1)):
                nc.tensor.matmul(ps[:],
                                 w3[:, cc, 128 * mb:128 * (mb + 1)],
                                 x_sl(cc, qb),
                                 start=(cc == 0), stop=(cc == 7))
            if part == nparts - 1:
                nc.vector.tensor_copy(
                    dst_sb[:, N * mb + 512 * qb:N * mb + 512 * (qb + 1)], ps[:])
                del qk_state[key]

        def qk_proj(w3, x_sl, dst_sb, mb, qb):
            qk_proj_part(w3, x_sl, dst_sb, mb, qb, 0, 1)

        def v_proj_block(tb):
            """V' for key-chunk tb: out[128 keys, 256] -> v1 cols 0:64."""
            ps = gpool.tile([128, 512], F32, tag="g", name=f"vps{tb}")
            for cc in range(8):
                nc.tensor.matmul(ps[:, 0:CB],
                                 xv3[tb // 8][:, cc, 128 * (tb % 8):
                                              128 * (tb % 8) + 128],
                                 wv3[:, cc, :],
                                 start=(cc == 0), stop=(cc == 7))
            nc.vector.tensor_copy(
                v3[:, tb, :, 0:D],
                ps[:, 0:CB].rearrange("p (h d) -> p h d", d=D))

        def pv_kc(hp, qb, kc):
            """PV chunk kc for both heads of hp; finishes at kc 15."""
            Pp = P_tiles[(hp, qb, kc)]
            for j in range(2):
                h = 2 * hp + j
                if kc == 0:
                    PV[(h, qb)] = pvpool.tile([128, 512], F32, tag="po",
                                              name=f"po{h}_{qb}")
                po = PV[(h, qb)]
                nc.tensor.matmul(po[0:DV, :],
                                 v3[:, kc, h, :],
                                 Pp[:, 512 * j:512 * (j + 1)],
                                 start=(kc == 0), stop=(kc == NCHUNK - 1))
            P_tiles.pop((hp, qb, kc))
            if kc == NCHUNK - 1:
                pv_finish(2 * hp, qb)
                pv_finish(2 * hp + 1, qb)

        def pv_finish(h, qb):
            """Normalize: A^T(h) = po[0:64] / po[64] -> aT plane.

            The raw numerators are staged straight into the aT plane and
            normalized IN PLACE, so po's PSUM bank frees after the copy +
            reciprocal (~0.9us) instead of after the whole broadcast
            chain (~2.5us) -- the next group's PV reuses that bank.
            """
            po = PV.pop((h, qb))
            dst = aT0_sb if h < 2 else aT1_sb
            sl = dst[64 * (h % 2):64 * (h % 2) + 64,
                     512 * qb:512 * (qb + 1)]
            stage = dpool.tile([64, 512], F32, tag="stg", name=f"st{h}_{qb}")
            dinv = dpool.tile([64, 512], F32, tag="dinv", name=f"di{h}_{qb}")
            nc.vector.tensor_copy(stage[:], po[0:D, :])
            nc.vector.tensor_copy(draw_sb[:], po[64:65, :])
            nc.vector.reciprocal_approx_fast(drow_sb[:], draw_sb[:])
            nc.gpsimd.partition_broadcast(dinv[:], drow_sb[:])
            nc.vector.tensor_mul(sl, stage[:], dinv[:])

        def oproj_m(qb, m, scalar_cast=False):
            """One m-block of the out-proj partial for query block qb."""
            ps = gpool.tile([128, 512], F32, tag="g", name=f"ops{m}_{qb}")
            for j in range(2):
                aT = (aT0_sb, aT1_sb)[j]
                nc.tensor.matmul(ps[:], wo3[:, j, 128 * m:128 * (m + 1)],
                                 aT[:, 512 * qb:512 * (qb + 1)],
                                 start=(j == 0), stop=(j == 1))
            ev = ospool.tile([128, 512], BF16, tag="ev", name=f"oev{m}_{qb}")
            if scalar_cast:
                nc.scalar.copy(ev[:], ps[:])
            else:
                nc.vector.tensor_copy(ev[:], ps[:])
            dq = nc.scalar if scalar_cast and qb == NQB - 1 else nc.sync
            dq.dma_start(
                out=outT[128 * m:128 * (m + 1), 512 * qb:512 * (qb + 1)],
                in_=ev[:])

        # ---------------- emission ----------------
        # Slot (hp, qb, pair): both score tiles FIRST (so the two exps
        # chain without waiting on rider matmuls), then PV of the lagged
        # group, then riders.  Riders are split small (<=4 matmuls) and
        # staggered so no slot carries a full 1.7us projection group.
        def run_group(hp, qb, riders, pv_src=None, pre=None):
            for pair in range(8):
                scores_kc(hp, qb, 2 * pair)
                scores_kc(hp, qb, 2 * pair + 1)
                for it in (pre or {}).get(pair, ()):
                    it()
                if pv_src is not None:
                    pv_kc(pv_src[0], pv_src[1], 2 * pair)
                    pv_kc(pv_src[0], pv_src[1], 2 * pair + 1)
                for it in riders.get(pair, ()):
                    it()

        def run_final_group():
            """(qb3, hp1): pv of (hp0, qb3) front-loaded on pairs 0-3 so
            its two PSUM tiles free up for (hp1, qb3)'s pv on pairs 4-7;
            all PV and the last exp finish in-slot, tail is just
            normalize + out-proj."""
            qb = NQB - 1
            for pair in range(8):
                hp_src, base = (0, 4 * pair) if pair < 4 else (1, 4 * (pair - 4))
                scores_kc(1, qb, 2 * pair)
                pv_kc(hp_src, qb, base)
                pv_kc(hp_src, qb, base + 1)
                scores_kc(1, qb, 2 * pair + 1)
                pv_kc(hp_src, qb, base + 2)
                pv_kc(hp_src, qb, base + 3)
                oproj_m(qb - 1, pair, scalar_cast=(pair >= 4))

        def kp(mb, kb, part, nparts=2):
            return lambda: qk_proj_part(wk3, xk_sl, kT_sb, mb, kb, part, nparts)

        def qp(mb, q, part, nparts=2):
            return lambda: qk_proj_part(wq3, xq_sl, qT_sb, mb, q, part, nparts)

        def vp(tb):
            return lambda: v_proj_block(tb)

        # Pre-loop: only what the first slot needs.
        qk_proj(wq3, xq_sl, qT_sb, 0, 0)
        qk_proj(wk3, xk_sl, kT_sb, 0, 0)

        # ---- group (qb0, hp0): K projections + first V' chunks ride ----
        # Deadlines: (0,kb) before pair 2*kb; (1,*) and Q(1,0) before the
        # hp1 group; v chunks 0,1 before hp1 pair 0.
        r0 = {
            0: [kp(1, 0, 0), kp(1, 0, 1)],
            1: [kp(0, 1, 0), kp(0, 1, 1)],
            2: [kp(1, 1, 0), kp(1, 1, 1)],
            3: [kp(0, 2, 0), kp(0, 2, 1)],
            4: [kp(1, 2, 0), kp(1, 2, 1)],
            5: [kp(0, 3, 0), kp(0, 3, 1)],
            6: [qp(1, 0, 0), qp(1, 0, 1)],
            7: [kp(1, 3, 0), kp(1, 3, 1), vp(0), vp(1)],
        }
        run_group(0, 0, r0)
        xk_stack.close()

        # ---- group (qb0, hp1): pv(qb0,hp0) + V' chunks + Q(qb1) ----
        # Slot p produces v chunks 2p+2,2p+3 (pre, ahead of the pv that
        # consumes them next slot) and consumes 2p,2p+1.
        pre1 = {p: [vp(2 * p + 2), vp(2 * p + 3)] for p in range(7)}
        r1 = {6: [qp(0, 1, 0)], 7: [qp(0, 1, 1)]}
        run_group(1, 0, r1, pv_src=(0, 0), pre=pre1)
        xv_stack.close()

        # ---- steady state: qb 1..3 ----
        for qb in range(1, NQB):
            # hp0: pv of (qb-1, hp1); Q proj (plane 1 of this qb) rides.
            r = {p: [qp(1, qb, p, 4)] for p in range(4)}
            run_group(0, qb, r, pv_src=(1, qb - 1))
            if qb == NQB - 1:
                xq_stack.close()
                run_final_group()
            else:
                # hp1: pv of (qb, hp0); out-proj of qb-1 and Q proj
                # (plane 0 of qb+1) ride.
                r = {p: [qp(0, qb + 1, p, 4)] for p in range(4)}
                for p in range(8):
                    r.setdefault(p, []).append(
                        lambda m=p, q=qb - 1: oproj_m(q, m))
                run_group(1, qb, r, pv_src=(0, qb))

        # ---- tail: normalize finished in final group; out-proj of qb3 ----
        jk2 = spool.tile([128, 1024], F32, tag="st", name="junk2")
        for i in range(14):
            nc.tensor.matmul(jk2[:, 0:512], v1_sb[:, 0:128], v1_sb[:, 0:512],
                             start=(i == 0), stop=(i == 13))
        for m in range(8):
            oproj_m(NQB - 1, m, scalar_cast=(m % 2 == 1))
        main.close()

    nc.compile()
    return nc


def _get_nc():
    if "nc" not in _CACHE:
        _CACHE["nc"] = build_nc()
    return _CACHE["nc"]


def _make_in_maps(q, k, v, Wq, Wk, Wv, Wo):
    bf = ml_dtypes.bfloat16
    q, k, v = np.asarray(q), np.asarray(k), np.asarray(v)
    Wq, Wk, Wv, Wo = (np.asarray(x) for x in (Wq, Wk, Wv, Wo))

    # x pre-shuffles: per-core SBUF wants partition p, cc-chunk c, col n
    # contiguous per transfer block.
    def xq_pre(xT):      # [C, N] -> [128, 4*4096], quarters of 512 tokens
        return np.ascontiguousarray(
            xT.reshape(8, 128, 4, 512).transpose(1, 2, 0, 3).reshape(128, -1)
        ).astype(bf)

    def xv_pre(xT):      # [C, N] -> [128, 2*8192], halves of 1024 keys
        return np.ascontiguousarray(
            xT.reshape(8, 128, 2, 1024).transpose(1, 2, 0, 3).reshape(128, -1)
        ).astype(bf)

    def w_pre(W):        # [1024, 256] -> [128, 2048]
        return np.ascontiguousarray(
            W.reshape(8, 128, 256).transpose(1, 0, 2).reshape(128, -1)
        ).astype(bf)

    def wo_pre(W):       # [256, 1024] -> [128, 2048]
        return np.ascontiguousarray(
            W.reshape(2, 128, 1024).transpose(1, 0, 2).reshape(128, -1)
        ).astype(bf)

    qT = [q[b].T for b in range(B)]
    kT = [k[b].T for b in range(B)]
    vT = [v[b].T for b in range(B)]
    in_maps = []
    for c in range(8):
        b, g = c // 4, c % 4
        cs = slice(CB * g, CB * (g + 1))
        in_maps.append({
            "xq": xq_pre(qT[b]), "xk": xq_pre(kT[b]), "xv": xv_pre(vT[b]),
            "wq": w_pre(Wq[:, cs]), "wk": w_pre(Wk[:, cs]),
            "wv": w_pre(Wv[:, cs]), "wo": wo_pre(Wo[cs, :]),
        })
    return in_maps


def _run(inputs, trace=False, **kw):
    nc = _get_nc()
    in_maps = _make_in_maps(inputs["q"], inputs["k"], inputs["v"],
                            inputs["Wq"], inputs["Wk"], inputs["Wv"], inputs["Wo"])
    res = None
    for attempt in range(3):
        try:
            res = run_bass_kernel_spmd(nc, in_maps, core_ids=list(range(8)),
                                       trace=trace, **kw)
            break
        except Exception:
            if attempt == 2:
                raise
            import time
            time.sleep(2.0)
    out = np.empty((B, N, C), np.float32)
    for b in range(B):
        acc = np.zeros((C, N), np.float32)
        for g in range(4):
            acc += res.results[4 * b + g]["outT"].astype(np.float32)
        out[b] = acc.T
    return out, res


def kernel(**inputs) -> np.ndarray:
    out, _ = _run(inputs, trace=False)
    return out
